# revision 5
# baseline (speedup 1.0000x reference)
"""GQA (B=2, S=2048, d_model=2048, 16 Q heads / 4 KV groups) + output projection.

Sharding: 8 cores, core c <-> (b = c//4, g = c%4). Each core computes full
attention for the 4 query heads of KV group g of batch b, then multiplies its
512-feature slice of the concatenated head outputs with the matching 512 rows
of Wc^T, producing a partial [S, d_model] projection (bf16). Host sums the 4
partials per batch element and adds the bias.

v3 vs baseline: all matmul operands are bf16 (0.4% rel err, budget 2e-2) and
the softmax-denominator matmuls (256 PE matmuls = 25% of baseline PE work) are
replaced by a DVE pairwise-add tree over the exp tiles (bf16, 2x mode) plus a
single gpsimd partition_all_reduce per combo (fp32-internal 128-way sum whose
output lands broadcast across all partitions, so no transpose/broadcast chain
is needed). The projection bias is added on the host during the partial-sum
gather. v2's DMA-XBAR-transpose sigma path was abandoned: concurrent
dma_start_transpose streams produce wrong data on hardware.

Per combo k = (j, h), j = 512-row s-block, h = head:
  scoresT[t, s] = kT.T @ qT           (PE, 16 matmuls, psum fp32)
  expT = exp(scoresT / sqrt(128))     (ACT, bf16 out, no max subtraction)
  tree: sum of 16 expT t-tiles        (DVE, 8 adds, bf16 2x)
  sigb[p, s] = all-partition sum      (GPSIMD partition_all_reduce, fp32 out)
  rbc = 1/sigb                        (DVE reciprocal_approx_fast, fp32)
  uT[hd, s]     = v.T @ expT          (PE, 16 matmuls, accumulated)
  attnT = uT * rbc                    (DVE, bf16 out)
  proj: po[s, o] = attnT.T @ wT       (PE, accumulate over the 4 heads)
  po -> orow bf16 copies split ACT/DVE, one out DMA per 128-row strip.
"""

import math
import sys

sys.path.insert(0, "/opt/trn_rl_repo")

import numpy as np
import ml_dtypes

import concourse.bacc as bacc
import concourse.bass as bass
import concourse.bass_isa as bass_isa
import concourse.mybir as mybir
import concourse.tile as tile
from concourse.bass import ds, ts
from concourse.bass_utils import run_bass_kernel_spmd

F32 = mybir.dt.float32
BF16 = mybir.dt.bfloat16

B = 2
S = 2048
D_MODEL = 2048
N_GROUPS = 4
HEADS_PER_GROUP = 4
HEAD_DIM = 128
P = 128
NT = S // P          # 16 t tiles
NJ = S // 512        # 4 s blocks
SCALE = 1.0 / math.sqrt(HEAD_DIM)

_COMPILED = None
DEBUG_ALL = False  # dump rbc/at for every combo


def _build():
    nc = bacc.Bacc(None, target_bir_lowering=False)

    qT_d = nc.dram_tensor("qT", [P, HEADS_PER_GROUP, S], BF16, kind="ExternalInput")
    kT_d = nc.dram_tensor("kT", [P, S], BF16, kind="ExternalInput")
    v_d = nc.dram_tensor("v", [P, NT, P], BF16, kind="ExternalInput")
    wT_d = nc.dram_tensor("wT", [P, HEADS_PER_GROUP, D_MODEL], BF16, kind="ExternalInput")
    out_d = nc.dram_tensor("out", [S, D_MODEL], BF16, kind="ExternalOutput")
    if DEBUG_ALL:
        dbg_rbca_d = nc.dram_tensor("dbg_rbca", [P, 16, 512], F32, kind="ExternalOutput")
        dbg_ata_d = nc.dram_tensor("dbg_ata", [P, 16, 512], BF16, kind="ExternalOutput")

    Exp = mybir.ActivationFunctionType.Exp
    Copy = mybir.ActivationFunctionType.Copy
    mult = mybir.AluOpType.mult
    add = mybir.AluOpType.add

    with tile.TileContext(nc) as tc:
        with (
            tc.tile_pool(name="const", bufs=1) as const_pool,
            tc.tile_pool(name="qt", bufs=3) as qt_pool,
            tc.tile_pool(name="expT", bufs=24) as expT_pool,
            tc.tile_pool(name="tr1", bufs=8) as tr1_pool,
            tc.tile_pool(name="tr2", bufs=4) as tr2_pool,
            tc.tile_pool(name="tr3", bufs=2) as tr3_pool,
            tc.tile_pool(name="sg", bufs=2) as sg_pool,
            tc.tile_pool(name="rbc", bufs=2) as rbc_pool,
            tc.tile_pool(name="attnT", bufs=8) as attnT_pool,
            tc.tile_pool(name="orow", bufs=2) as orow_pool,
            tc.tile_pool(name="qk_ps", bufs=2, space="PSUM") as qk_psum,
            tc.tile_pool(name="pv_ps", bufs=2, space="PSUM") as pv_psum,
            tc.tile_pool(name="po_ps", bufs=2, space="PSUM") as po_psum,
        ):
            # first QK dependency chain on the sync queue; bulky v/wT go on
            # GPSIMD SWDGE so they don't serialize behind it
            kT_sb = const_pool.tile([P, S], BF16, tag="kT")
            nc.sync.dma_start(kT_sb[:, ts(0, 512)], kT_d[:, ts(0, 512)])
            qt0 = qt_pool.tile([P, 512], BF16, tag="qT")
            nc.sync.dma_start(qt0[:], qT_d[:, 0, ts(0, 512)])
            for cc in range(1, 4):
                nc.sync.dma_start(kT_sb[:, ts(cc, 512)], kT_d[:, ts(cc, 512)])
            v_sb = const_pool.tile([P, NT, P], BF16, tag="v")
            nc.gpsimd.dma_start(v_sb[:], v_d[:])
            wT_sb = const_pool.tile([P, HEADS_PER_GROUP, D_MODEL], BF16, tag="wT")

            attnT_tiles = {}

            def emit_qk_pv(k, prev_pairs):
                # Interleave QK pairs of combo k with PV pairs of combo k-1 in
                # the PE stream: the PV matmuls cover the exp latency so the
                # qk-psum double buffer never stalls the PE.
                pairs = None
                if k is not None:
                    j, h = divmod(k, HEADS_PER_GROUP)
                    if k == 0:
                        qt = qt0
                    else:
                        qt = qt_pool.tile([P, 512], BF16, tag="qT")
                        nc.sync.dma_start(qt[:], qT_d[:, h, ts(j, 512)])
                    pairs = []
                pv_ps = None
                if prev_pairs is not None:
                    pv_ps = pv_psum.tile([P, 512], F32, tag="pv")
                for pp in range(NT // 2):
                    if pairs is not None:
                        ps = qk_psum.tile([P, 2, 512], F32, tag="qk")
                        et = expT_pool.tile([P, 2, 512], BF16, tag="expT")
                        for u in range(2):
                            tt = pp * 2 + u
                            nc.tensor.matmul(
                                ps[:, u, :], kT_sb[:, ts(tt, P)], qt[:],
                                start=True, stop=True,
                            )
                        nc.scalar.activation(et[:], ps[:], Exp, scale=SCALE)
                        pairs.append(et)
                    if pv_ps is not None:
                        pet = prev_pairs[pp]
                        for u in range(2):
                            tt = pp * 2 + u
                            nc.tensor.matmul(
                                pv_ps[:], v_sb[:, tt, :], pet[:, u, :],
                                start=(tt == 0), stop=(tt == NT - 1),
                            )
                return pairs, pv_ps

            def emit_sigma(k, pairs):
                # denominators: bf16 pairwise tree over the 8 exp pair-tiles
                # (DVE 2x), then one gpsimd all-partition fp32 sum that lands
                # already broadcast over all 128 partitions
                t1 = []
                for i in range(4):
                    t = tr1_pool.tile([P, 2, 512], BF16, tag="t1")
                    nc.vector.tensor_tensor(t[:], pairs[2 * i][:], pairs[2 * i + 1][:], add)
                    t1.append(t)
                t2 = []
                for i in range(2):
                    t = tr2_pool.tile([P, 2, 512], BF16, tag="t2")
                    nc.vector.tensor_tensor(t[:], t1[2 * i][:], t1[2 * i + 1][:], add)
                    t2.append(t)
                t3 = tr3_pool.tile([P, 2, 512], BF16, tag="t3")
                nc.vector.tensor_tensor(t3[:], t2[0][:], t2[1][:], add)
                s1 = tr3_pool.tile([P, 512], BF16, tag="s1")
                nc.vector.tensor_tensor(s1[:], t3[:, 0, :], t3[:, 1, :], add)
                sigb = sg_pool.tile([P, 512], F32, tag="sigb")
                nc.gpsimd.partition_all_reduce(
                    sigb[:], s1[:], channels=P, reduce_op=bass_isa.ReduceOp.add
                )
                rbc = rbc_pool.tile([P, 512], F32, tag="rbc")
                nc.vector.reciprocal_approx_fast(rbc[:], sigb[:])
                if DEBUG_ALL:
                    nc.sync.dma_start(dbg_rbca_d[:, k, :], rbc[:])
                return rbc

            def emit_norm(k, pv_ps, rbc):
                j, h = divmod(k, HEADS_PER_GROUP)
                at = attnT_pool.tile([P, 512], BF16, tag="attnT")
                nc.vector.tensor_tensor(at[:], pv_ps[:], rbc[:], mult)
                if DEBUG_ALL:
                    nc.scalar.dma_start(dbg_ata_d[:, k, :], at[:])
                attnT_tiles[(j, h)] = at

            def emit_proj_row(j, st):
                # one 128-row strip: 4 output blocks x 4 head-contraction
                # matmuls; psum->sbuf copies rotate 1:3 over ACT/DVE (gpsimd
                # can't read PSUM)
                orow = orow_pool.tile([P, NJ, 512], BF16, tag="orow")
                for ob in range(4):
                    po = po_psum.tile([P, 512], F32, tag="po")
                    for h in range(HEADS_PER_GROUP):
                        nc.tensor.matmul(
                            po[:], attnT_tiles[(j, h)][:, ts(st, P)],
                            wT_sb[:, h, ts(ob, 512)],
                            start=(h == 0), stop=(h == HEADS_PER_GROUP - 1),
                        )
                    if ob % 4 == 0:
                        nc.scalar.activation(orow[:, ob, :], po[:], Copy)
                    else:
                        nc.vector.tensor_copy(orow[:, ob, :], po[:])
                nc.sync.dma_start(out_d[ds(j * 512 + st * P, P), :], orow[:])

            n_combos = NJ * HEADS_PER_GROUP
            # proj rows per iteration: (j, st) strips; j=3's strips doubled up
            # at the tail so the drain is 2 iterations, not 4
            proj_sched = {}
            for j in range(NJ):
                for r in range(4):
                    it = 4 * j + 5 + r if j < 3 else 17 + r // 2
                    proj_sched.setdefault(it, []).append((j, r))
            prev = None
            for k in range(n_combos + 3):
                # proj row first: its psum->sbuf copies land early in the
                # DVE/ACT queues so the po WAR never stalls the PE
                for j, r in proj_sched.get(k, []):
                    emit_proj_row(j, r)
                rbc = None
                if 1 <= k <= n_combos:
                    # sigma chain next so the DVE tree and gpsimd all-reduce
                    # overlap this iteration's PE work
                    rbc = emit_sigma(k - 1, prev)
                pairs, pv_ps = emit_qk_pv(
                    k if k < n_combos else None, prev
                )
                if k == 0:
                    # wT is only needed by proj (first use ~50us in)
                    nc.gpsimd.dma_start(wT_sb[:], wT_d[:])
                if 1 <= k <= n_combos:
                    emit_norm(k - 1, pv_ps, rbc)
                if k < n_combos:
                    prev = pairs
                else:
                    prev = None

    nc.compile()
    return nc


def _get_nc():
    global _COMPILED
    if _COMPILED is None:
        _COMPILED = _build()
    return _COMPILED


def _bf(x):
    return np.ascontiguousarray(x).astype(ml_dtypes.bfloat16)


def _shard_inputs(q, k, v, Wc):
    in_maps = []
    for c in range(8):
        b, g = divmod(c, 4)
        qT = _bf(
            q[b][:, g * 512:(g + 1) * 512]
            .reshape(S, HEADS_PER_GROUP, P).transpose(2, 1, 0)
        )
        kT = _bf(k[b][:, g * P:(g + 1) * P].T)
        vv = _bf(v[b][:, g * P:(g + 1) * P].reshape(NT, P, P).transpose(1, 0, 2))
        wT = _bf(Wc[:, g * 512:(g + 1) * 512].T.reshape(HEADS_PER_GROUP, P, D_MODEL).transpose(1, 0, 2))
        in_maps.append({"qT": qT, "kT": kT, "v": vv, "wT": wT})
    return in_maps


def _run(inputs, trace=False):
    q = np.asarray(inputs["q"], dtype=np.float32)
    k = np.asarray(inputs["k"], dtype=np.float32)
    v = np.asarray(inputs["v"], dtype=np.float32)
    Wc = np.asarray(inputs["Wc"], dtype=np.float32)
    bc = np.asarray(inputs["bc"], dtype=np.float32)

    nc = _get_nc()
    in_maps = _shard_inputs(q, k, v, Wc)
    res = run_bass_kernel_spmd(nc, in_maps, list(range(8)), trace=trace)

    out = np.empty((B, S, D_MODEL), dtype=np.float32)
    for b in range(B):
        acc = res.results[4 * b]["out"].astype(np.float32)
        for g in range(1, 4):
            acc = acc + res.results[4 * b + g]["out"].astype(np.float32)
        out[b] = acc + bc.reshape(1, D_MODEL)
    return out, res


def kernel(**inputs):
    out, _ = _run(inputs, trace=False)
    return out


# revision 6
# speedup vs baseline: 1.0044x; 1.0044x over previous
"""GQA (B=2, S=2048, d_model=2048, 16 Q heads / 4 KV groups) + output projection.

Sharding: 8 cores, core c <-> (b = c//4, g = c%4). Each core computes full
attention for the 4 query heads of KV group g of batch b, then multiplies its
512-feature slice of the concatenated head outputs with the matching 512 rows
of Wc^T, producing a partial [S, d_model] projection (bf16). Host sums the 4
partials per batch element and adds the bias.

v3 vs baseline: all matmul operands are bf16 (0.4% rel err, budget 2e-2) and
the softmax-denominator matmuls (256 PE matmuls = 25% of baseline PE work) are
replaced by a DVE pairwise-add tree over the exp tiles (bf16, 2x mode) plus a
single gpsimd partition_all_reduce per combo (fp32-internal 128-way sum whose
output lands broadcast across all partitions, so no transpose/broadcast chain
is needed). The projection bias is added on the host during the partial-sum
gather. v2's DMA-XBAR-transpose sigma path was abandoned: concurrent
dma_start_transpose streams produce wrong data on hardware.

Per combo k = (j, h), j = 512-row s-block, h = head:
  scoresT[t, s] = kT.T @ qT           (PE, 16 matmuls, psum fp32)
  expT = exp(scoresT / sqrt(128))     (ACT, bf16 out, no max subtraction)
  tree: sum of 16 expT t-tiles        (DVE, 8 adds, bf16 2x)
  sigb[p, s] = all-partition sum      (GPSIMD partition_all_reduce, fp32 out)
  rbc = 1/sigb                        (DVE reciprocal_approx_fast, fp32)
  uT[hd, s]     = v.T @ expT          (PE, 16 matmuls, accumulated)
  attnT = uT * rbc                    (DVE, bf16 out)
  proj: po[s, o] = attnT.T @ wT       (PE, accumulate over the 4 heads)
  po -> orow bf16 copies split ACT/DVE, one out DMA per 128-row strip.
"""

import math
import sys

sys.path.insert(0, "/opt/trn_rl_repo")

import numpy as np
import ml_dtypes

import concourse.bacc as bacc
import concourse.bass as bass
import concourse.bass_isa as bass_isa
import concourse.mybir as mybir
import concourse.tile as tile
from concourse.bass import ds, ts
from concourse.bass_utils import run_bass_kernel_spmd

F32 = mybir.dt.float32
BF16 = mybir.dt.bfloat16

B = 2
S = 2048
D_MODEL = 2048
N_GROUPS = 4
HEADS_PER_GROUP = 4
HEAD_DIM = 128
P = 128
NT = S // P          # 16 t tiles
NJ = S // 512        # 4 s blocks
SCALE = 1.0 / math.sqrt(HEAD_DIM)

_COMPILED = None
DEBUG_ALL = False  # dump rbc/at for every combo


def _build():
    nc = bacc.Bacc(None, target_bir_lowering=False)

    qT_d = nc.dram_tensor("qT", [P, HEADS_PER_GROUP, S], BF16, kind="ExternalInput")
    kT_d = nc.dram_tensor("kT", [P, S], BF16, kind="ExternalInput")
    v_d = nc.dram_tensor("v", [P, NT, P], BF16, kind="ExternalInput")
    wT_d = nc.dram_tensor("wT", [P, HEADS_PER_GROUP, D_MODEL], BF16, kind="ExternalInput")
    out_d = nc.dram_tensor("out", [S, D_MODEL], BF16, kind="ExternalOutput")
    if DEBUG_ALL:
        dbg_rbca_d = nc.dram_tensor("dbg_rbca", [P, 16, 512], F32, kind="ExternalOutput")
        dbg_ata_d = nc.dram_tensor("dbg_ata", [P, 16, 512], BF16, kind="ExternalOutput")

    Exp = mybir.ActivationFunctionType.Exp
    Copy = mybir.ActivationFunctionType.Copy
    mult = mybir.AluOpType.mult
    add = mybir.AluOpType.add

    with tile.TileContext(nc) as tc:
        with (
            tc.tile_pool(name="const", bufs=1) as const_pool,
            tc.tile_pool(name="qt", bufs=3) as qt_pool,
            tc.tile_pool(name="expT", bufs=24) as expT_pool,
            tc.tile_pool(name="tr1", bufs=8) as tr1_pool,
            tc.tile_pool(name="tr2", bufs=4) as tr2_pool,
            tc.tile_pool(name="tr3", bufs=2) as tr3_pool,
            tc.tile_pool(name="sg", bufs=2) as sg_pool,
            tc.tile_pool(name="rbc", bufs=2) as rbc_pool,
            tc.tile_pool(name="attnT", bufs=8) as attnT_pool,
            tc.tile_pool(name="orow", bufs=2) as orow_pool,
            tc.tile_pool(name="qk_ps", bufs=2, space="PSUM") as qk_psum,
            tc.tile_pool(name="pv_ps", bufs=2, space="PSUM") as pv_psum,
            tc.tile_pool(name="po_ps", bufs=2, space="PSUM") as po_psum,
        ):
            # first QK dependency chain on the sync queue; bulky v/wT go on
            # GPSIMD SWDGE so they don't serialize behind it
            kT_sb = const_pool.tile([P, S], BF16, tag="kT")
            nc.sync.dma_start(kT_sb[:, ts(0, 512)], kT_d[:, ts(0, 512)])
            qt0 = qt_pool.tile([P, 512], BF16, tag="qT")
            nc.sync.dma_start(qt0[:], qT_d[:, 0, ts(0, 512)])
            for cc in range(1, 4):
                nc.sync.dma_start(kT_sb[:, ts(cc, 512)], kT_d[:, ts(cc, 512)])
            v_sb = const_pool.tile([P, NT, P], BF16, tag="v")
            nc.gpsimd.dma_start(v_sb[:], v_d[:])
            wT_sb = const_pool.tile([P, HEADS_PER_GROUP, D_MODEL], BF16, tag="wT")

            attnT_tiles = {}
            sigb_tiles = {}

            def emit_qk_pv(k, prev_pairs):
                # Interleave QK pairs of combo k with PV pairs of combo k-1 in
                # the PE stream (the PV matmuls cover the exp latency so the
                # qk-psum double buffer never stalls the PE), and weave combo
                # k's sigma tree into the same iteration: level-1 adds fire as
                # each exp pair completes, tree tail + gpsimd all-reduce at
                # the end. The reciprocal runs early next iteration, so the
                # normalize never sits at the end of a 10us chain that would
                # stall PV(k+1) on the pv-psum WAR.
                pairs = None
                tree1 = []
                if k is not None:
                    j, h = divmod(k, HEADS_PER_GROUP)
                    if k == 0:
                        qt = qt0
                    else:
                        qt = qt_pool.tile([P, 512], BF16, tag="qT")
                        nc.sync.dma_start(qt[:], qT_d[:, h, ts(j, 512)])
                    pairs = []
                pv_ps = None
                if prev_pairs is not None:
                    pv_ps = pv_psum.tile([P, 512], F32, tag="pv")
                for pp in range(NT // 2):
                    if pairs is not None:
                        ps = qk_psum.tile([P, 2, 512], F32, tag="qk")
                        et = expT_pool.tile([P, 2, 512], BF16, tag="expT")
                        for u in range(2):
                            tt = pp * 2 + u
                            nc.tensor.matmul(
                                ps[:, u, :], kT_sb[:, ts(tt, P)], qt[:],
                                start=True, stop=True,
                            )
                        nc.scalar.activation(et[:], ps[:], Exp, scale=SCALE)
                        pairs.append(et)
                    if pv_ps is not None:
                        pet = prev_pairs[pp]
                        for u in range(2):
                            tt = pp * 2 + u
                            nc.tensor.matmul(
                                pv_ps[:], v_sb[:, tt, :], pet[:, u, :],
                                start=(tt == 0), stop=(tt == NT - 1),
                            )
                    if pairs is not None and pp % 2 == 1:
                        t = tr1_pool.tile([P, 2, 512], BF16, tag="t1")
                        nc.vector.tensor_tensor(
                            t[:], pairs[pp - 1][:], pairs[pp][:], add
                        )
                        tree1.append(t)
                if pairs is not None:
                    t2 = []
                    for i in range(2):
                        t = tr2_pool.tile([P, 2, 512], BF16, tag="t2")
                        nc.vector.tensor_tensor(
                            t[:], tree1[2 * i][:], tree1[2 * i + 1][:], add
                        )
                        t2.append(t)
                    t3 = tr3_pool.tile([P, 2, 512], BF16, tag="t3")
                    nc.vector.tensor_tensor(t3[:], t2[0][:], t2[1][:], add)
                    s1 = tr3_pool.tile([P, 512], BF16, tag="s1")
                    nc.vector.tensor_tensor(s1[:], t3[:, 0, :], t3[:, 1, :], add)
                    sigb = sg_pool.tile([P, 512], F32, tag="sigb")
                    nc.gpsimd.partition_all_reduce(
                        sigb[:], s1[:], channels=P, reduce_op=bass_isa.ReduceOp.add
                    )
                    sigb_tiles[k] = sigb
                return pairs, pv_ps

            def emit_recip(k):
                rbc = rbc_pool.tile([P, 512], F32, tag="rbc")
                nc.vector.reciprocal_approx_fast(rbc[:], sigb_tiles.pop(k)[:])
                if DEBUG_ALL:
                    nc.sync.dma_start(dbg_rbca_d[:, k, :], rbc[:])
                return rbc

            def emit_norm(k, pv_ps, rbc):
                j, h = divmod(k, HEADS_PER_GROUP)
                at = attnT_pool.tile([P, 512], BF16, tag="attnT")
                nc.vector.tensor_tensor(at[:], pv_ps[:], rbc[:], mult)
                if DEBUG_ALL:
                    nc.scalar.dma_start(dbg_ata_d[:, k, :], at[:])
                attnT_tiles[(j, h)] = at

            def emit_proj_row(j, st):
                # one 128-row strip: 4 output blocks x 4 head-contraction
                # matmuls; psum->sbuf copies rotate 1:3 over ACT/DVE (gpsimd
                # can't read PSUM)
                orow = orow_pool.tile([P, NJ, 512], BF16, tag="orow")
                for ob in range(4):
                    po = po_psum.tile([P, 512], F32, tag="po")
                    for h in range(HEADS_PER_GROUP):
                        nc.tensor.matmul(
                            po[:], attnT_tiles[(j, h)][:, ts(st, P)],
                            wT_sb[:, h, ts(ob, 512)],
                            start=(h == 0), stop=(h == HEADS_PER_GROUP - 1),
                        )
                    if ob % 4 == 0:
                        nc.scalar.activation(orow[:, ob, :], po[:], Copy)
                    else:
                        nc.vector.tensor_copy(orow[:, ob, :], po[:])
                nc.sync.dma_start(out_d[ds(j * 512 + st * P, P), :], orow[:])

            n_combos = NJ * HEADS_PER_GROUP
            # proj rows per iteration: (j, st) strips; j=3's strips doubled up
            # at the tail so the drain is 2 iterations, not 4
            proj_sched = {}
            for j in range(NJ):
                for r in range(4):
                    it = 4 * j + 5 + r if j < 3 else 17 + r // 2
                    proj_sched.setdefault(it, []).append((j, r))
            prev = None
            for k in range(n_combos + 3):
                # proj row first: its psum->sbuf copies land early in the
                # DVE/ACT queues so the po WAR never stalls the PE
                for j, r in proj_sched.get(k, []):
                    emit_proj_row(j, r)
                rbc = None
                if 1 <= k <= n_combos:
                    rbc = emit_recip(k - 1)
                pairs, pv_ps = emit_qk_pv(
                    k if k < n_combos else None, prev
                )
                if k == 0:
                    # wT is only needed by proj (first use ~50us in)
                    nc.gpsimd.dma_start(wT_sb[:], wT_d[:])
                if 1 <= k <= n_combos:
                    emit_norm(k - 1, pv_ps, rbc)
                if k < n_combos:
                    prev = pairs
                else:
                    prev = None

    nc.compile()
    return nc


def _get_nc():
    global _COMPILED
    if _COMPILED is None:
        _COMPILED = _build()
    return _COMPILED


def _bf(x):
    return np.ascontiguousarray(x).astype(ml_dtypes.bfloat16)


def _shard_inputs(q, k, v, Wc):
    in_maps = []
    for c in range(8):
        b, g = divmod(c, 4)
        qT = _bf(
            q[b][:, g * 512:(g + 1) * 512]
            .reshape(S, HEADS_PER_GROUP, P).transpose(2, 1, 0)
        )
        kT = _bf(k[b][:, g * P:(g + 1) * P].T)
        vv = _bf(v[b][:, g * P:(g + 1) * P].reshape(NT, P, P).transpose(1, 0, 2))
        wT = _bf(Wc[:, g * 512:(g + 1) * 512].T.reshape(HEADS_PER_GROUP, P, D_MODEL).transpose(1, 0, 2))
        in_maps.append({"qT": qT, "kT": kT, "v": vv, "wT": wT})
    return in_maps


def _run(inputs, trace=False):
    q = np.asarray(inputs["q"], dtype=np.float32)
    k = np.asarray(inputs["k"], dtype=np.float32)
    v = np.asarray(inputs["v"], dtype=np.float32)
    Wc = np.asarray(inputs["Wc"], dtype=np.float32)
    bc = np.asarray(inputs["bc"], dtype=np.float32)

    nc = _get_nc()
    in_maps = _shard_inputs(q, k, v, Wc)
    res = run_bass_kernel_spmd(nc, in_maps, list(range(8)), trace=trace)

    out = np.empty((B, S, D_MODEL), dtype=np.float32)
    for b in range(B):
        acc = res.results[4 * b]["out"].astype(np.float32)
        for g in range(1, 4):
            acc = acc + res.results[4 * b + g]["out"].astype(np.float32)
        out[b] = acc + bc.reshape(1, D_MODEL)
    return out, res


def kernel(**inputs):
    out, _ = _run(inputs, trace=False)
    return out


# revision 7
# speedup vs baseline: 1.0081x; 1.0037x over previous
"""GQA (B=2, S=2048, d_model=2048, 16 Q heads / 4 KV groups) + output projection.

Sharding: 8 cores, core c <-> (b = c//4, g = c%4). Each core computes full
attention for the 4 query heads of KV group g of batch b, then multiplies its
512-feature slice of the concatenated head outputs with the matching 512 rows
of Wc^T, producing a partial [S, d_model] projection (bf16). Host sums the 4
partials per batch element and adds the bias.

v3 vs baseline: all matmul operands are bf16 (0.4% rel err, budget 2e-2) and
the softmax-denominator matmuls (256 PE matmuls = 25% of baseline PE work) are
replaced by a DVE pairwise-add tree over the exp tiles (bf16, 2x mode) plus a
single gpsimd partition_all_reduce per combo (fp32-internal 128-way sum whose
output lands broadcast across all partitions, so no transpose/broadcast chain
is needed). The projection bias is added on the host during the partial-sum
gather. v2's DMA-XBAR-transpose sigma path was abandoned: concurrent
dma_start_transpose streams produce wrong data on hardware.

Per combo k = (j, h), j = 512-row s-block, h = head:
  scoresT[t, s] = kT.T @ qT           (PE, 16 matmuls, psum fp32)
  expT = exp(scoresT / sqrt(128))     (ACT, bf16 out, no max subtraction)
  tree: sum of 16 expT t-tiles        (DVE, 8 adds, bf16 2x)
  sigb[p, s] = all-partition sum      (GPSIMD partition_all_reduce, fp32 out)
  rbc = 1/sigb                        (DVE reciprocal_approx_fast, fp32)
  uT[hd, s]     = v.T @ expT          (PE, 16 matmuls, accumulated)
  attnT = uT * rbc                    (DVE, bf16 out)
  proj: po[s, o] = attnT.T @ wT       (PE, accumulate over the 4 heads)
  po -> orow bf16 copies split ACT/DVE, one out DMA per 128-row strip.
"""

import math
import sys

sys.path.insert(0, "/opt/trn_rl_repo")

import numpy as np
import ml_dtypes

import concourse.bacc as bacc
import concourse.bass as bass
import concourse.bass_isa as bass_isa
import concourse.mybir as mybir
import concourse.tile as tile
from concourse.bass import ds, ts
from concourse.bass_utils import run_bass_kernel_spmd

F32 = mybir.dt.float32
BF16 = mybir.dt.bfloat16

B = 2
S = 2048
D_MODEL = 2048
N_GROUPS = 4
HEADS_PER_GROUP = 4
HEAD_DIM = 128
P = 128
NT = S // P          # 16 t tiles
NJ = S // 512        # 4 s blocks
SCALE = 1.0 / math.sqrt(HEAD_DIM)

_COMPILED = None
DEBUG_ALL = False  # dump rbc/at for every combo


def _build():
    nc = bacc.Bacc(None, target_bir_lowering=False)

    qT_d = nc.dram_tensor("qT", [P, HEADS_PER_GROUP, S], BF16, kind="ExternalInput")
    kT_d = nc.dram_tensor("kT", [P, S], BF16, kind="ExternalInput")
    v_d = nc.dram_tensor("v", [P, NT, P], BF16, kind="ExternalInput")
    wT_d = nc.dram_tensor("wT", [P, HEADS_PER_GROUP, D_MODEL], BF16, kind="ExternalInput")
    out_d = nc.dram_tensor("out", [S, D_MODEL], BF16, kind="ExternalOutput")
    if DEBUG_ALL:
        dbg_rbca_d = nc.dram_tensor("dbg_rbca", [P, 16, 512], F32, kind="ExternalOutput")
        dbg_ata_d = nc.dram_tensor("dbg_ata", [P, 16, 512], BF16, kind="ExternalOutput")

    Exp = mybir.ActivationFunctionType.Exp
    Copy = mybir.ActivationFunctionType.Copy
    mult = mybir.AluOpType.mult
    add = mybir.AluOpType.add

    with tile.TileContext(nc) as tc:
        with (
            tc.tile_pool(name="const", bufs=1) as const_pool,
            tc.tile_pool(name="qt", bufs=3) as qt_pool,
            tc.tile_pool(name="expT", bufs=24) as expT_pool,
            tc.tile_pool(name="tr1", bufs=8) as tr1_pool,
            tc.tile_pool(name="tr2", bufs=4) as tr2_pool,
            tc.tile_pool(name="tr3", bufs=2) as tr3_pool,
            tc.tile_pool(name="sg", bufs=2) as sg_pool,
            tc.tile_pool(name="rbc", bufs=2) as rbc_pool,
            tc.tile_pool(name="attnT", bufs=8) as attnT_pool,
            tc.tile_pool(name="orow", bufs=2) as orow_pool,
            tc.tile_pool(name="qk_ps", bufs=2, space="PSUM") as qk_psum,
            tc.tile_pool(name="pv_ps", bufs=2, space="PSUM") as pv_psum,
            tc.tile_pool(name="po_ps", bufs=2, space="PSUM") as po_psum,
        ):
            # first QK dependency chain on the sync queue; bulky v/wT go on
            # GPSIMD SWDGE so they don't serialize behind it
            kT_sb = const_pool.tile([P, S], BF16, tag="kT")
            nc.sync.dma_start(kT_sb[:, ts(0, 512)], kT_d[:, ts(0, 512)])
            qt0 = qt_pool.tile([P, 512], BF16, tag="qT")
            nc.sync.dma_start(qt0[:], qT_d[:, 0, ts(0, 512)])
            for cc in range(1, 4):
                nc.sync.dma_start(kT_sb[:, ts(cc, 512)], kT_d[:, ts(cc, 512)])
            v_sb = const_pool.tile([P, NT, P], BF16, tag="v")
            nc.gpsimd.dma_start(v_sb[:], v_d[:])
            wT_sb = const_pool.tile([P, HEADS_PER_GROUP, D_MODEL], BF16, tag="wT")

            attnT_tiles = {}
            sigb_tiles = {}

            def emit_qk_pv(k, prev_pairs):
                # Interleave QK pairs of combo k with PV pairs of combo k-1 in
                # the PE stream (the PV matmuls cover the exp latency so the
                # qk-psum double buffer never stalls the PE), and weave combo
                # k's sigma tree into the same iteration: level-1 adds fire as
                # each exp pair completes, tree tail + gpsimd all-reduce at
                # the end. The reciprocal runs early next iteration, so the
                # normalize never sits at the end of a 10us chain that would
                # stall PV(k+1) on the pv-psum WAR.
                pairs = None
                tree1 = []
                if k is not None:
                    j, h = divmod(k, HEADS_PER_GROUP)
                    if k == 0:
                        qt = qt0
                    else:
                        qt = qt_pool.tile([P, 512], BF16, tag="qT")
                        nc.sync.dma_start(qt[:], qT_d[:, h, ts(j, 512)])
                    pairs = []
                pv_ps = None
                if prev_pairs is not None:
                    pv_ps = pv_psum.tile([P, 512], F32, tag="pv")
                for pp in range(NT // 2):
                    if pairs is not None:
                        ps = qk_psum.tile([P, 2, 512], F32, tag="qk")
                        et = expT_pool.tile([P, 2, 512], BF16, tag="expT")
                        for u in range(2):
                            tt = pp * 2 + u
                            nc.tensor.matmul(
                                ps[:, u, :], kT_sb[:, ts(tt, P)], qt[:],
                                start=True, stop=True,
                            )
                        nc.scalar.activation(et[:], ps[:], Exp, scale=SCALE)
                        pairs.append(et)
                    if pv_ps is not None:
                        pet = prev_pairs[pp]
                        for u in range(2):
                            tt = pp * 2 + u
                            nc.tensor.matmul(
                                pv_ps[:], v_sb[:, tt, :], pet[:, u, :],
                                start=(tt == 0), stop=(tt == NT - 1),
                            )
                    if pairs is not None and pp % 2 == 1 and pp < 7:
                        t = tr1_pool.tile([P, 2, 512], BF16, tag="t1")
                        nc.vector.tensor_tensor(
                            t[:], pairs[pp - 1][:], pairs[pp][:], add
                        )
                        tree1.append(t)
                return pairs, pv_ps, tree1

            def emit_tree_tail(k, pairs, tree1):
                # emitted after norm(k-1) so the normalize isn't queued behind
                # the level-1 add that waits on this combo's last exp pair
                t = tr1_pool.tile([P, 2, 512], BF16, tag="t1")
                nc.vector.tensor_tensor(t[:], pairs[6][:], pairs[7][:], add)
                tree1.append(t)
                t2 = []
                for i in range(2):
                    t = tr2_pool.tile([P, 2, 512], BF16, tag="t2")
                    nc.vector.tensor_tensor(
                        t[:], tree1[2 * i][:], tree1[2 * i + 1][:], add
                    )
                    t2.append(t)
                t3 = tr3_pool.tile([P, 2, 512], BF16, tag="t3")
                nc.vector.tensor_tensor(t3[:], t2[0][:], t2[1][:], add)
                s1 = tr3_pool.tile([P, 512], BF16, tag="s1")
                nc.vector.tensor_tensor(s1[:], t3[:, 0, :], t3[:, 1, :], add)
                sigb = sg_pool.tile([P, 512], F32, tag="sigb")
                nc.gpsimd.partition_all_reduce(
                    sigb[:], s1[:], channels=P, reduce_op=bass_isa.ReduceOp.add
                )
                sigb_tiles[k] = sigb

            def emit_recip(k):
                rbc = rbc_pool.tile([P, 512], F32, tag="rbc")
                nc.vector.reciprocal_approx_fast(rbc[:], sigb_tiles.pop(k)[:])
                if DEBUG_ALL:
                    nc.sync.dma_start(dbg_rbca_d[:, k, :], rbc[:])
                return rbc

            def emit_norm(k, pv_ps, rbc):
                j, h = divmod(k, HEADS_PER_GROUP)
                at = attnT_pool.tile([P, 512], BF16, tag="attnT")
                nc.vector.tensor_tensor(at[:], pv_ps[:], rbc[:], mult)
                if DEBUG_ALL:
                    nc.scalar.dma_start(dbg_ata_d[:, k, :], at[:])
                attnT_tiles[(j, h)] = at

            def emit_proj_row(j, st):
                # one 128-row strip: 4 output blocks x 4 head-contraction
                # matmuls; psum->sbuf copies rotate 1:3 over ACT/DVE (gpsimd
                # can't read PSUM)
                orow = orow_pool.tile([P, NJ, 512], BF16, tag="orow")
                for ob in range(4):
                    po = po_psum.tile([P, 512], F32, tag="po")
                    for h in range(HEADS_PER_GROUP):
                        nc.tensor.matmul(
                            po[:], attnT_tiles[(j, h)][:, ts(st, P)],
                            wT_sb[:, h, ts(ob, 512)],
                            start=(h == 0), stop=(h == HEADS_PER_GROUP - 1),
                        )
                    if ob % 4 == 0:
                        nc.scalar.activation(orow[:, ob, :], po[:], Copy)
                    else:
                        nc.vector.tensor_copy(orow[:, ob, :], po[:])
                nc.sync.dma_start(out_d[ds(j * 512 + st * P, P), :], orow[:])

            n_combos = NJ * HEADS_PER_GROUP
            # proj rows per iteration: (j, st) strips; j=3's strips doubled up
            # at the tail so the drain is 2 iterations, not 4
            proj_sched = {}
            for j in range(NJ):
                for r in range(4):
                    it = 4 * j + 5 + r if j < 3 else 17 + r // 2
                    proj_sched.setdefault(it, []).append((j, r))
            prev = None
            for k in range(n_combos + 3):
                # proj row first: its psum->sbuf copies land early in the
                # DVE/ACT queues so the po WAR never stalls the PE
                for j, r in proj_sched.get(k, []):
                    emit_proj_row(j, r)
                rbc = None
                if 1 <= k <= n_combos:
                    rbc = emit_recip(k - 1)
                pairs, pv_ps, tree1 = emit_qk_pv(
                    k if k < n_combos else None, prev
                )
                if k == 0:
                    # wT is only needed by proj (first use ~50us in)
                    nc.gpsimd.dma_start(wT_sb[:], wT_d[:])
                if 1 <= k <= n_combos:
                    emit_norm(k - 1, pv_ps, rbc)
                if k < n_combos:
                    emit_tree_tail(k, pairs, tree1)
                    prev = pairs
                else:
                    prev = None

    nc.compile()
    return nc


def _get_nc():
    global _COMPILED
    if _COMPILED is None:
        _COMPILED = _build()
    return _COMPILED


def _bf(x):
    return np.ascontiguousarray(x).astype(ml_dtypes.bfloat16)


def _shard_inputs(q, k, v, Wc):
    in_maps = []
    for c in range(8):
        b, g = divmod(c, 4)
        qT = _bf(
            q[b][:, g * 512:(g + 1) * 512]
            .reshape(S, HEADS_PER_GROUP, P).transpose(2, 1, 0)
        )
        kT = _bf(k[b][:, g * P:(g + 1) * P].T)
        vv = _bf(v[b][:, g * P:(g + 1) * P].reshape(NT, P, P).transpose(1, 0, 2))
        wT = _bf(Wc[:, g * 512:(g + 1) * 512].T.reshape(HEADS_PER_GROUP, P, D_MODEL).transpose(1, 0, 2))
        in_maps.append({"qT": qT, "kT": kT, "v": vv, "wT": wT})
    return in_maps


def _run(inputs, trace=False):
    q = np.asarray(inputs["q"], dtype=np.float32)
    k = np.asarray(inputs["k"], dtype=np.float32)
    v = np.asarray(inputs["v"], dtype=np.float32)
    Wc = np.asarray(inputs["Wc"], dtype=np.float32)
    bc = np.asarray(inputs["bc"], dtype=np.float32)

    nc = _get_nc()
    in_maps = _shard_inputs(q, k, v, Wc)
    res = run_bass_kernel_spmd(nc, in_maps, list(range(8)), trace=trace)

    out = np.empty((B, S, D_MODEL), dtype=np.float32)
    for b in range(B):
        acc = res.results[4 * b]["out"].astype(np.float32)
        for g in range(1, 4):
            acc = acc + res.results[4 * b + g]["out"].astype(np.float32)
        out[b] = acc + bc.reshape(1, D_MODEL)
    return out, res


def kernel(**inputs):
    out, _ = _run(inputs, trace=False)
    return out
